# revision 4
# baseline (speedup 1.0000x reference)
"""Trainium2 Bass kernel for ChannelwiseSlidingWindowDropout2D.

Reference semantics (see problem):
    bits  = (noise < 0.1)                      # [C, 58, 58]
    drop  = maxpool7x7(bits, pad=(6,6))        # [C, 64, 64]
    out   = x * (1 - drop)[None]               # [B, C, H, W], mask batch-shared

Formulation used here (exact for the mask, bf16 rounding on x only):
    keep[c,y,x] = 1.0  iff  every noise value in the 7x7 window covering
    (y,x) is >= 0.1; out = x * keep.

Sharding: channels split across the 8 cores (32 channels per core). Each
core receives x[:, c0:c0+32] cast to bf16 and flattened to [1024, 64, 64]
plus its noise slice padded with 1.0 along W to [58, 70] (fp32 - the
0.1 comparison must be exact). x and y move over HBM as bf16, halving
the DMA traffic that bounds this kernel; the 2e-2 rel-err budget dwarfs
bf16's ~0.4% rounding.

Mask pipeline per core:
  1. keep-bits KB = (noise >= 0.1) as 0/1 in bf16 (exact) on the DVE.
  2. W-direction erosion (min over 7) by shift-doubling 1->2->4->7 on the
     DVE; bits are 0/1 so bf16 min is exact.
  3. H-direction erosion FUSED with the 32->128 partition broadcast on the
     otherwise-idle tensor engine: for 0/1 bits, min over 7 rows == (sum
     of 7 shifted rows == 7). Seven accumulating matmuls per PSUM bank
     with the 0/1 replication matrix R (R[k,p] = p%32==k) compute
     S[p,(h,w)] = sum_dy WB[p%32, h+dy, w] in {0..7}.
  4. keep = relu(S - 6) in {0,1} (exact: S is integer-valued) on the
     otherwise-idle scalar engine, which also casts PSUM fp32 -> bf16.
  5. DVE multiplies each [128, 32, 64] bf16 x-tile by the mask (2x DVE
     mode: 16-bit, unit stride, 4B-aligned) and the result streams out.

DMA: x loads on the sync HWDGE ring; noise/R + most stores on the scalar
HWDGE ring; the last four stores ride the sync ring once its loads drain.
The mask work is split in H halves so stores start ~20us in, keeping the
kernel at the bf16 HBM roofline (~17.3 MB/core / 358 GB/s ~ 48us).
"""

import numpy as np

B, C, H, W = 32, 256, 64, 64
WIN = 7
DROP_PROB = 0.1
HV, WV = H - WIN + 1, W - WIN + 1  # 58, 58
N_CORES = 8
C_PER_CORE = C // N_CORES  # 32
ROWS = B * C_PER_CORE      # 1024 rows of [64, 64] per core
PAD = H + WIN - 1          # 70: 1.0-padded plane side
ROW_TILES = ROWS // 128    # 8 partition-tiles per core
H_SPLIT = 2
HS = H // H_SPLIT          # 32
WCHUNK = 33                # W-pass rows [0,33) cover the h=0 mask half

_CACHE = {}


def _build():
    import concourse.tile as tile
    from concourse import bacc, mybir
    import concourse.bass as bass

    f32 = mybir.dt.float32
    bf16 = mybir.dt.bfloat16
    op_min = mybir.AluOpType.min
    op_mul = mybir.AluOpType.mult
    op_ge = mybir.AluOpType.is_ge
    relu = mybir.ActivationFunctionType.Relu

    nc = bacc.Bacc("TRN2", target_bir_lowering=False, debug=False)

    x_d = nc.declare_dram_parameter("xs", [ROWS, H, W], bf16, isOutput=False)
    n_d = nc.declare_dram_parameter("np", [C_PER_CORE, HV * PAD], f32, isOutput=False)
    r_d = nc.declare_dram_parameter("rp", [C_PER_CORE, 128], bf16, isOutput=False)
    y_d = nc.declare_dram_parameter("y", [ROWS, H, W], bf16, isOutput=True)

    CP = C_PER_CORE

    with tile.TileContext(nc) as tc:
        with (
            tc.tile_pool(name="tpool", bufs=1) as tpool,
            tc.tile_pool(name="xpool", bufs=16) as xpool,
            tc.tile_pool(name="ppool", bufs=1, space=bass.MemorySpace.PSUM) as ppool,
        ):
            # noise + replication matrix ride the scalar (store) ring so the
            # sync ring starts streaming x at t=0
            P = tpool.tile([CP, HV, PAD], f32, tag="P")  # [32p, 58, 70]
            nc.scalar.dma_start(out=P[:], in_=n_d[:])
            R = tpool.tile([CP, 128], bf16, tag="R")
            nc.scalar.dma_start(out=R[:], in_=r_d[:])

            # x loads, h-major so the h=0 multiply pass never waits on a load
            xts = {}
            for h in range(H_SPLIT):
                for t in range(ROW_TILES):
                    xt = xpool.tile([128, HS, W], bf16, tag="xt", name=f"xt{t}_{h}")
                    nc.sync.dma_start(
                        out=xt[:],
                        in_=x_d[128 * t : 128 * (t + 1), h * HS : (h + 1) * HS, :],
                    )
                    xts[(t, h)] = xt

            KB = tpool.tile([CP, HV, PAD], bf16, tag="KB")      # keep bits
            T1 = tpool.tile([CP, HV, PAD - 1], bf16, tag="T1")  # [58, 69]
            T2 = tpool.tile([CP, HV, PAD - 3], bf16, tag="T2")  # [58, 67]
            WB = tpool.tile([CP, PAD, W], bf16, tag="WB")       # [70, 64]
            # H-padding rows have no input deps: memset during the noise DMA
            nc.vector.memset(WB[:, 0 : WIN - 1, :], 1.0)
            nc.vector.memset(WB[:, WIN - 1 + HV :, :], 1.0)

            def w_pass(lo, hi):
                # keep-bits + W-erosion for noise rows [lo, hi) -> WB rows
                # [6+lo, 6+hi). Rows are independent; chunked so the h=0
                # PE stage starts before the whole plane is eroded.
                nc.vector.tensor_scalar(
                    out=KB[:, lo:hi, :], in0=P[:, lo:hi, :],
                    scalar1=DROP_PROB, scalar2=None, op0=op_ge,
                )
                nc.vector.tensor_tensor(
                    out=T1[:, lo:hi, :], in0=KB[:, lo:hi, 0:69],
                    in1=KB[:, lo:hi, 1:70], op=op_min,
                )
                nc.vector.tensor_tensor(
                    out=T2[:, lo:hi, :], in0=T1[:, lo:hi, 0:67],
                    in1=T1[:, lo:hi, 2:69], op=op_min,
                )
                nc.vector.tensor_tensor(
                    out=WB[:, 6 + lo : 6 + hi, :], in0=T2[:, lo:hi, 0:64],
                    in1=T2[:, lo:hi, 3:67], op=op_min,
                )

            S = ppool.tile([128, H, W], f32)           # all 8 PSUM banks
            MB = tpool.tile([128, H, W], bf16, tag="MB")  # 0/1 keep mask
            BIAS = tpool.tile([128, 1], f32, tag="bias")  # relu(S - 6) bias
            nc.vector.memset(BIAS[:], -6.0)

            def h_pe(h):
                # H-erosion + broadcast: one PSUM bank per 8 output rows,
                # 7 accumulating matmuls per bank (one per row shift)
                for j in range(4):
                    jj = 4 * h + j
                    for dy in range(WIN):
                        nc.tensor.matmul(
                            out=S[:, 8 * jj : 8 * (jj + 1), :],
                            lhsT=R[:],
                            rhs=WB[:, 8 * jj + dy : 8 * jj + dy + 8, :],
                            start=(dy == 0),
                            stop=(dy == WIN - 1),
                        )

            def relu_mask(h):
                # keep = relu(S - 6): exact 0/1, PSUM fp32 -> SBUF bf16
                nc.scalar.activation(
                    out=MB[:, h * HS : (h + 1) * HS, :],
                    in_=S[:, h * HS : (h + 1) * HS, :],
                    func=relu, bias=BIAS[:], scale=1.0,
                )

            def mul_store(t, h):
                xt = xts[(t, h)]
                nc.vector.tensor_tensor(
                    out=xt[:], in0=xt[:],
                    in1=MB[:, h * HS : (h + 1) * HS, :], op=op_mul,
                )
                # tail stores ride the sync ring (its loads are done by then)
                eng = nc.sync if (h == 1 and t >= ROW_TILES // 2) else nc.scalar
                eng.dma_start(
                    out=y_d[128 * t : 128 * (t + 1), h * HS : (h + 1) * HS, :],
                    in_=xt[:],
                )

            w_pass(0, WCHUNK)
            h_pe(0)
            relu_mask(0)
            w_pass(WCHUNK, HV)
            h_pe(1)
            for t in range(ROW_TILES):
                mul_store(t, 0)
            relu_mask(1)
            for t in range(ROW_TILES):
                mul_store(t, 1)

    nc.compile()
    return nc


def _get_nc():
    if "nc" not in _CACHE:
        _CACHE["nc"] = _build()
    return _CACHE["nc"]


def _pad_noise(noise_slice: np.ndarray) -> np.ndarray:
    """[32, 58, 58] -> [32, 58*70]: pad W with 1.0 to [58, 70] (interior at
    [:, 6:64]). H-padding rows are supplied on-chip by the WB memsets."""
    p = np.ones((C_PER_CORE, HV, PAD), dtype=np.float32)
    p[:, :, WIN - 1 : WIN - 1 + WV] = noise_slice
    return p.reshape(C_PER_CORE, HV * PAD)


def _repl_matrix() -> np.ndarray:
    """[32, 128] 0/1 matrix with R[k, p] = (p % 32 == k): R.T @ m replicates
    a 32-partition tensor onto 128 partitions (p reads row p % 32)."""
    r = np.zeros((C_PER_CORE, 128), dtype=np.float32)
    cols = np.arange(128)
    r[cols % C_PER_CORE, cols] = 1.0
    return r


def kernel(x: np.ndarray, noise: np.ndarray) -> np.ndarray:
    from concourse.bass_utils import run_bass_kernel_spmd
    import ml_dtypes

    bf = ml_dtypes.bfloat16
    x = np.asarray(x, dtype=np.float32)
    noise = np.asarray(noise, dtype=np.float32)

    nc = _get_nc()
    xb = x.astype(bf)
    rp = _repl_matrix().astype(bf)
    in_maps = []
    for i in range(N_CORES):
        c0 = i * C_PER_CORE
        xs = np.ascontiguousarray(xb[:, c0 : c0 + C_PER_CORE]).reshape(ROWS, H, W)
        ns = _pad_noise(noise[c0 : c0 + C_PER_CORE])
        in_maps.append({"xs": xs, "np": ns, "rp": rp})

    res = run_bass_kernel_spmd(nc, in_maps, core_ids=list(range(N_CORES)))
    _CACHE["last_results"] = res

    out = np.empty((B, C, H, W), dtype=np.float32)
    for i in range(N_CORES):
        c0 = i * C_PER_CORE
        out[:, c0 : c0 + C_PER_CORE] = res.results[i]["y"].reshape(
            B, C_PER_CORE, H, W
        )
    return out


# revision 6
# speedup vs baseline: 1.0445x; 1.0445x over previous
"""Trainium2 Bass kernel for ChannelwiseSlidingWindowDropout2D.

Reference semantics (see problem):
    bits  = (noise < 0.1)                      # [C, 58, 58]
    drop  = maxpool7x7(bits, pad=(6,6))        # [C, 64, 64]
    out   = x * (1 - drop)[None]               # [B, C, H, W], mask batch-shared

Formulation used here (mask exact, bf16 rounding on x only):
    keep[c,y,x] = 1.0 iff every noise value in the 7x7 window covering
    (y,x) is >= 0.1; out = x * keep. keep-bits = (noise >= 0.1) are
    computed on the host (the 0.1 comparison must be fp32-exact; bits are
    0/1 so bf16 min-erosion of them on device is exact).

Sharding: channels split across the 8 cores (32 channels per core). x and
y move over HBM as bf16, halving the DMA traffic that bounds this kernel;
the 2e-2 rel-err budget dwarfs bf16's ~0.4% rounding.

Mask pipeline per core (dilation == erosion of keep-bits, all on device):
  1. Host lays keep-bits out QUARTERED: partition 32g+c holds rows
     [16g, 16g+22) of channel c's 70x70 1.0-padded bit plane (6-row halo
     so each quarter erodes independently). All 128 partitions then run
     the separable 7x7 min-erosion in ~7us of DVE time: W-shifts
     1->2->4->7 by doubling, then H-shifts on rows (even strides keep the
     DVE in 2x bf16 mode for most ops).
  2. The eroded quarter-masks M4[32g+c] = keep rows [16g,16g+16) are
     gathered+broadcast to the x layout (partition p <- channel p%32) on
     the idle tensor engine: per 8-row PSUM bank one matmul with the
     0/1 replication matrix R4 (R4[p,q] = q%32==p%32), exact for bits.
  3. The idle scalar engine copies PSUM fp32 -> SBUF bf16 per quarter.
  4. DVE (plus two trial tiles on GpSimd) multiplies each [128, 32, 64]
     bf16 x-tile by its mask half (2x DVE mode) and the result streams
     out as bf16.

DMA: x loads and y stores are split across BOTH HWDGE rings (sync +
scalar) so neither ring idles; bits/R4 lead on the scalar ring. Roofline:
~17 MB/core of HBM traffic.
"""

import numpy as np

B, C, H, W = 32, 256, 64, 64
WIN = 7
DROP_PROB = 0.1
HV, WV = H - WIN + 1, W - WIN + 1  # 58, 58
N_CORES = 8
C_PER_CORE = C // N_CORES  # 32
ROWS = B * C_PER_CORE      # 1024 rows of [64, 64] per core
PAD = H + WIN - 1          # 70: 1.0-padded bit-plane side
ROW_TILES = ROWS // 128    # 8 partition-tiles per core
H_SPLIT = 2
HS = H // H_SPLIT          # 32
QROWS = 16                 # output rows per quarter
QIN = QROWS + WIN - 1      # 22: input rows per quarter (with halo)

_CACHE = {}


def _build():
    import concourse.tile as tile
    from concourse import bacc, mybir
    import concourse.bass as bass

    f32 = mybir.dt.float32
    bf16 = mybir.dt.bfloat16
    op_min = mybir.AluOpType.min
    op_mul = mybir.AluOpType.mult
    copy_fn = mybir.ActivationFunctionType.Copy

    nc = bacc.Bacc("TRN2", target_bir_lowering=False, debug=False)

    x_d = nc.declare_dram_parameter("xs", [ROWS, H, W], bf16, isOutput=False)
    b_d = nc.declare_dram_parameter("bq", [128, QIN * PAD], bf16, isOutput=False)
    r_d = nc.declare_dram_parameter("rp", [128, 128], bf16, isOutput=False)
    y_d = nc.declare_dram_parameter("y", [ROWS, H, W], bf16, isOutput=True)

    with tile.TileContext(nc) as tc:
        with (
            tc.tile_pool(name="tpool", bufs=1) as tpool,
            tc.tile_pool(name="xpool", bufs=16) as xpool,
            tc.tile_pool(name="ppool", bufs=1, space=bass.MemorySpace.PSUM) as ppool,
        ):
            # bits + replication matrix lead on the scalar ring; x loads
            # start at t=0 on both rings (even tiles sync, odd scalar)
            B4 = tpool.tile([128, QIN, PAD], bf16, tag="B4")  # quartered bits
            nc.scalar.dma_start(out=B4[:], in_=b_d[:])
            R4 = tpool.tile([128, 128], bf16, tag="R4")
            nc.scalar.dma_start(out=R4[:], in_=r_d[:])

            xts = {}
            for h in range(H_SPLIT):
                for t in range(ROW_TILES):
                    xt = xpool.tile([128, HS, W], bf16, tag="xt", name=f"xt{t}_{h}")
                    eng = nc.sync if t % 2 == 0 else nc.scalar
                    eng.dma_start(
                        out=xt[:],
                        in_=x_d[128 * t : 128 * (t + 1), h * HS : (h + 1) * HS, :],
                    )
                    xts[(t, h)] = xt

            # separable 7-point min-erosion, all quarters in parallel
            T1 = tpool.tile([128, QIN, PAD - 1], bf16, tag="T1")  # [22, 69]
            T2 = tpool.tile([128, QIN, PAD - 3], bf16, tag="T2")  # [22, 67]
            W7 = tpool.tile([128, QIN, W], bf16, tag="W7")        # [22, 64]
            U1 = tpool.tile([128, QIN - 1, W], bf16, tag="U1")    # [21, 64]
            U2 = tpool.tile([128, QIN - 3, W], bf16, tag="U2")    # [19, 64]
            M4 = tpool.tile([128, QROWS, W], bf16, tag="M4")      # [16, 64]
            nc.vector.tensor_tensor(
                out=T1[:], in0=B4[:, :, 0:69], in1=B4[:, :, 1:70], op=op_min
            )
            nc.vector.tensor_tensor(
                out=T2[:], in0=T1[:, :, 0:67], in1=T1[:, :, 2:69], op=op_min
            )
            nc.vector.tensor_tensor(
                out=W7[:], in0=T2[:, :, 0:64], in1=T2[:, :, 3:67], op=op_min
            )
            nc.vector.tensor_tensor(
                out=U1[:], in0=W7[:, 0:21, :], in1=W7[:, 1:22, :], op=op_min
            )
            nc.vector.tensor_tensor(
                out=U2[:], in0=U1[:, 0:19, :], in1=U1[:, 2:21, :], op=op_min
            )
            nc.vector.tensor_tensor(
                out=M4[:], in0=U2[:, 0:16, :], in1=U2[:, 3:19, :], op=op_min
            )

            # gather quarters to the x layout: MB[p, 16g:16g+16, :] holds
            # channel p%32's keep rows; one matmul per 8-row PSUM bank
            S = ppool.tile([128, H, W], f32)
            MB = tpool.tile([128, H, W], bf16, tag="MB")
            for g in range(4):
                for j in range(2):
                    r0 = 16 * g + 8 * j
                    nc.tensor.matmul(
                        out=S[:, r0 : r0 + 8, :],
                        lhsT=R4[32 * g : 32 * (g + 1), :],
                        rhs=M4[32 * g : 32 * (g + 1), 8 * j : 8 * j + 8, :],
                        start=True,
                        stop=True,
                        tile_position=(32 * g, 0),
                    )
                nc.scalar.activation(
                    out=MB[:, 16 * g : 16 * (g + 1), :],
                    in_=S[:, 16 * g : 16 * (g + 1), :],
                    func=copy_fn,
                )

            def mul_store(t, h, mul_eng):
                xt = xts[(t, h)]
                mul_eng.tensor_tensor(
                    out=xt[:], in0=xt[:],
                    in1=MB[:, h * HS : (h + 1) * HS, :], op=op_mul,
                )
                eng = nc.scalar if t % 2 == 0 else nc.sync
                eng.dma_start(
                    out=y_d[128 * t : 128 * (t + 1), h * HS : (h + 1) * HS, :],
                    in_=xt[:],
                )

            # tile 0 of each half rides GpSimd (trial: measures its TT cost
            # off the DVE critical path); the rest stream on the DVE
            for h in range(H_SPLIT):
                mul_store(0, h, nc.gpsimd)
            for h in range(H_SPLIT):
                for t in range(1, ROW_TILES):
                    mul_store(t, h, nc.vector)

    nc.compile()
    return nc


def _get_nc():
    if "nc" not in _CACHE:
        _CACHE["nc"] = _build()
    return _CACHE["nc"]


def _quartered_bits(noise_slice: np.ndarray, bf) -> np.ndarray:
    """[32, 58, 58] noise -> [128, 22*70] quartered keep-bit planes.

    PK[c] is the 70x70 1.0-padded keep-bit plane of channel c (interior
    [6:64, 6:64] = noise >= 0.1, fp32-exact on host). Partition 32g + c
    gets PK[c] rows [16g, 16g+22): output rows [16g, 16g+16) plus the
    6-row erosion halo.
    """
    pk = np.ones((C_PER_CORE, PAD, PAD), dtype=np.float32)
    pk[:, WIN - 1 : WIN - 1 + HV, WIN - 1 : WIN - 1 + WV] = (
        noise_slice >= DROP_PROB
    )
    b4 = np.empty((128, QIN, PAD), dtype=bf)
    for g in range(4):
        b4[32 * g : 32 * (g + 1)] = pk[:, QROWS * g : QROWS * g + QIN, :]
    return b4.reshape(128, QIN * PAD)


def _repl_matrix() -> np.ndarray:
    """[128, 128] 0/1 matrix with R4[p, q] = (q % 32 == p % 32)."""
    r = np.zeros((128, 128), dtype=np.float32)
    p, q = np.meshgrid(np.arange(128), np.arange(128), indexing="ij")
    r[(q % C_PER_CORE) == (p % C_PER_CORE)] = 1.0
    return r


def kernel(x: np.ndarray, noise: np.ndarray) -> np.ndarray:
    from concourse.bass_utils import run_bass_kernel_spmd
    import ml_dtypes

    bf = ml_dtypes.bfloat16
    x = np.asarray(x, dtype=np.float32)
    noise = np.asarray(noise, dtype=np.float32)

    nc = _get_nc()
    xb = x.astype(bf)
    rp = _repl_matrix().astype(bf)
    in_maps = []
    for i in range(N_CORES):
        c0 = i * C_PER_CORE
        xs = np.ascontiguousarray(xb[:, c0 : c0 + C_PER_CORE]).reshape(ROWS, H, W)
        bq = _quartered_bits(noise[c0 : c0 + C_PER_CORE], bf)
        in_maps.append({"xs": xs, "bq": bq, "rp": rp})

    res = run_bass_kernel_spmd(nc, in_maps, core_ids=list(range(N_CORES)))
    _CACHE["last_results"] = res

    out = np.empty((B, C, H, W), dtype=np.float32)
    for i in range(N_CORES):
        c0 = i * C_PER_CORE
        out[:, c0 : c0 + C_PER_CORE] = res.results[i]["y"].reshape(
            B, C_PER_CORE, H, W
        )
    return out


# revision 8
# speedup vs baseline: 1.0916x; 1.0451x over previous
"""Trainium2 Bass kernel for ChannelwiseSlidingWindowDropout2D.

Reference semantics (see problem):
    bits  = (noise < 0.1)                      # [C, 58, 58]
    drop  = maxpool7x7(bits, pad=(6,6))        # [C, 64, 64]
    out   = x * (1 - drop)[None]               # [B, C, H, W], mask batch-shared

Formulation used here (mask exact, bf16 rounding on x only):
    keep[c,y,x] = 1.0 iff every noise value in the 7x7 window covering
    (y,x) is >= 0.1; out = x * keep. keep-bits = (noise >= 0.1) are
    computed on the host (the 0.1 comparison must be fp32-exact; bits are
    0/1 so bf16 min-erosion of them on device is exact).

Sharding: channels split across the 8 cores (32 channels per core). x and
y move over HBM as bf16, halving the DMA traffic that bounds this kernel;
the 2e-2 rel-err budget dwarfs bf16's ~0.4% rounding.

Mask pipeline per core (dilation == erosion of keep-bits, all on device):
  1. Host lays keep-bits out QUARTERED: partition 32g+c holds rows
     [16g, 16g+22) of channel c's 70x70 1.0-padded bit plane (6-row halo
     so each quarter erodes independently). All 128 partitions then run
     the separable 7x7 min-erosion in ~7us of DVE time: W-shifts
     1->2->4->7 by doubling, then H-shifts on rows (even strides keep the
     DVE in 2x bf16 mode for most ops).
  2. The eroded quarter-masks M4[32g+c] = keep rows [16g,16g+16) are
     gathered+broadcast to the x layout (partition p <- channel p%32) on
     the idle tensor engine: per 8-row PSUM bank one matmul with the
     0/1 replication matrix R4 (R4[p,q] = q%32==p%32), exact for bits.
  3. The idle scalar engine copies PSUM fp32 -> SBUF bf16 per quarter.
  4. DVE (plus two trial tiles on GpSimd) multiplies each [128, 32, 64]
     bf16 x-tile by its mask half (2x DVE mode) and the result streams
     out as bf16.

DMA: x loads and y stores are split across BOTH HWDGE rings (sync +
scalar) so neither ring idles; bits/R4 lead on the scalar ring. Roofline:
~17 MB/core of HBM traffic.
"""

import numpy as np

B, C, H, W = 32, 256, 64, 64
WIN = 7
DROP_PROB = 0.1
HV, WV = H - WIN + 1, W - WIN + 1  # 58, 58
N_CORES = 8
C_PER_CORE = C // N_CORES  # 32
ROWS = B * C_PER_CORE      # 1024 rows of [64, 64] per core
PAD = H + WIN - 1          # 70: 1.0-padded bit-plane side
ROW_TILES = ROWS // 128    # 8 partition-tiles per core
H_SPLIT = 2
HS = H // H_SPLIT          # 32
QROWS = 16                 # output rows per quarter
QIN = QROWS + WIN - 1      # 22: input rows per quarter (with halo)

_CACHE = {}


def _build():
    import concourse.tile as tile
    from concourse import bacc, mybir
    import concourse.bass as bass

    f32 = mybir.dt.float32
    bf16 = mybir.dt.bfloat16
    op_min = mybir.AluOpType.min
    op_mul = mybir.AluOpType.mult
    copy_fn = mybir.ActivationFunctionType.Copy

    nc = bacc.Bacc("TRN2", target_bir_lowering=False, debug=False)

    x_d = nc.declare_dram_parameter("xs", [ROWS, H, W], bf16, isOutput=False)
    b_d = nc.declare_dram_parameter("bq", [128, QIN * PAD], bf16, isOutput=False)
    r_d = nc.declare_dram_parameter("rp", [128, 128], bf16, isOutput=False)
    y_d = nc.declare_dram_parameter("y", [ROWS, H, W], bf16, isOutput=True)

    with tile.TileContext(nc) as tc:
        with (
            tc.tile_pool(name="tpool", bufs=1) as tpool,
            tc.tile_pool(name="xpool", bufs=16) as xpool,
            tc.tile_pool(name="ppool", bufs=1, space=bass.MemorySpace.PSUM) as ppool,
        ):
            # bits + replication matrix lead on the scalar ring; x loads
            # start at t=0 on both rings (even tiles sync, odd scalar)
            B4 = tpool.tile([128, QIN, PAD], bf16, tag="B4")  # quartered bits
            nc.scalar.dma_start(out=B4[:], in_=b_d[:])
            R4 = tpool.tile([128, 128], bf16, tag="R4")
            nc.scalar.dma_start(out=R4[:], in_=r_d[:])

            # h=0 loads on the sync HWDGE ring, h=1 loads on the GpSimd
            # SWDGE ring: the scalar engine queue stays free of load issues
            # (a queued dma_start stalls the engine queue when the ring is
            # full, which in v3 delayed the mask copies by ~7us)
            xts = {}
            for h in range(H_SPLIT):
                for t in range(ROW_TILES):
                    xt = xpool.tile([128, HS, W], bf16, tag="xt", name=f"xt{t}_{h}")
                    eng = nc.sync if h == 0 else nc.gpsimd
                    eng.dma_start(
                        out=xt[:],
                        in_=x_d[128 * t : 128 * (t + 1), h * HS : (h + 1) * HS, :],
                    )
                    xts[(t, h)] = xt

            # separable 7-point min-erosion, all quarters in parallel
            T1 = tpool.tile([128, QIN, PAD - 1], bf16, tag="T1")  # [22, 69]
            T2 = tpool.tile([128, QIN, PAD - 3], bf16, tag="T2")  # [22, 67]
            W7 = tpool.tile([128, QIN, W], bf16, tag="W7")        # [22, 64]
            U1 = tpool.tile([128, QIN - 1, W], bf16, tag="U1")    # [21, 64]
            U2 = tpool.tile([128, QIN - 3, W], bf16, tag="U2")    # [19, 64]
            M4 = tpool.tile([128, QROWS, W], bf16, tag="M4")      # [16, 64]
            nc.vector.tensor_tensor(
                out=T1[:], in0=B4[:, :, 0:69], in1=B4[:, :, 1:70], op=op_min
            )
            nc.vector.tensor_tensor(
                out=T2[:], in0=T1[:, :, 0:67], in1=T1[:, :, 2:69], op=op_min
            )
            nc.vector.tensor_tensor(
                out=W7[:], in0=T2[:, :, 0:64], in1=T2[:, :, 3:67], op=op_min
            )
            nc.vector.tensor_tensor(
                out=U1[:], in0=W7[:, 0:21, :], in1=W7[:, 1:22, :], op=op_min
            )
            nc.vector.tensor_tensor(
                out=U2[:], in0=U1[:, 0:19, :], in1=U1[:, 2:21, :], op=op_min
            )
            nc.vector.tensor_tensor(
                out=M4[:], in0=U2[:, 0:16, :], in1=U2[:, 3:19, :], op=op_min
            )

            # gather quarters to the x layout: MB[p, 16g:16g+16, :] holds
            # channel p%32's keep rows; one matmul per 8-row PSUM bank
            S = ppool.tile([128, H, W], f32)
            MB = tpool.tile([128, H, W], bf16, tag="MB")
            for g in range(4):
                for j in range(2):
                    r0 = 16 * g + 8 * j
                    nc.tensor.matmul(
                        out=S[:, r0 : r0 + 8, :],
                        lhsT=R4[32 * g : 32 * (g + 1), :],
                        rhs=M4[32 * g : 32 * (g + 1), 8 * j : 8 * j + 8, :],
                        start=True,
                        stop=True,
                        tile_position=(32 * g, 0),
                    )
                nc.scalar.activation(
                    out=MB[:, 16 * g : 16 * (g + 1), :],
                    in_=S[:, 16 * g : 16 * (g + 1), :],
                    func=copy_fn,
                )

            def mul_store(t, h):
                xt = xts[(t, h)]
                nc.vector.tensor_tensor(
                    out=xt[:], in0=xt[:],
                    in1=MB[:, h * HS : (h + 1) * HS, :], op=op_mul,
                )
                eng = nc.scalar if t % 2 == 0 else nc.sync
                eng.dma_start(
                    out=y_d[128 * t : 128 * (t + 1), h * HS : (h + 1) * HS, :],
                    in_=xt[:],
                )

            # all multiplies on the DVE: a concurrent GpSimd tensor op was
            # measured to slow DVE tensor_tensor ~4x (SBUF contention)
            for h in range(H_SPLIT):
                for t in range(ROW_TILES):
                    mul_store(t, h)

    nc.compile()
    return nc


def _get_nc():
    if "nc" not in _CACHE:
        _CACHE["nc"] = _build()
    return _CACHE["nc"]


def _quartered_bits(noise_slice: np.ndarray, bf) -> np.ndarray:
    """[32, 58, 58] noise -> [128, 22*70] quartered keep-bit planes.

    PK[c] is the 70x70 1.0-padded keep-bit plane of channel c (interior
    [6:64, 6:64] = noise >= 0.1, fp32-exact on host). Partition 32g + c
    gets PK[c] rows [16g, 16g+22): output rows [16g, 16g+16) plus the
    6-row erosion halo.
    """
    pk = np.ones((C_PER_CORE, PAD, PAD), dtype=np.float32)
    pk[:, WIN - 1 : WIN - 1 + HV, WIN - 1 : WIN - 1 + WV] = (
        noise_slice >= DROP_PROB
    )
    b4 = np.empty((128, QIN, PAD), dtype=bf)
    for g in range(4):
        b4[32 * g : 32 * (g + 1)] = pk[:, QROWS * g : QROWS * g + QIN, :]
    return b4.reshape(128, QIN * PAD)


def _repl_matrix() -> np.ndarray:
    """[128, 128] 0/1 matrix with R4[p, q] = (q % 32 == p % 32)."""
    r = np.zeros((128, 128), dtype=np.float32)
    p, q = np.meshgrid(np.arange(128), np.arange(128), indexing="ij")
    r[(q % C_PER_CORE) == (p % C_PER_CORE)] = 1.0
    return r


def kernel(x: np.ndarray, noise: np.ndarray) -> np.ndarray:
    from concourse.bass_utils import run_bass_kernel_spmd
    import ml_dtypes

    bf = ml_dtypes.bfloat16
    x = np.asarray(x, dtype=np.float32)
    noise = np.asarray(noise, dtype=np.float32)

    nc = _get_nc()
    xb = x.astype(bf)
    rp = _repl_matrix().astype(bf)
    in_maps = []
    for i in range(N_CORES):
        c0 = i * C_PER_CORE
        xs = np.ascontiguousarray(xb[:, c0 : c0 + C_PER_CORE]).reshape(ROWS, H, W)
        bq = _quartered_bits(noise[c0 : c0 + C_PER_CORE], bf)
        in_maps.append({"xs": xs, "bq": bq, "rp": rp})

    res = run_bass_kernel_spmd(nc, in_maps, core_ids=list(range(N_CORES)))
    _CACHE["last_results"] = res

    out = np.empty((B, C, H, W), dtype=np.float32)
    for i in range(N_CORES):
        c0 = i * C_PER_CORE
        out[:, c0 : c0 + C_PER_CORE] = res.results[i]["y"].reshape(
            B, C_PER_CORE, H, W
        )
    return out
